# revision 1
# baseline (speedup 1.0000x reference)
"""CharDecoder LSTM (L=64, B=2048, H=1024, V=256) as a Bass/Tile kernel on 8 trn2 cores.

Strategy: data-parallel over batch (256 per core), weights replicated.
All on-chip state is kept in [feature, batch] (transposed) layout so the
recurrent matmul needs no transposes:
  gates^T[j, b] = sum_k W_hh^T[k, j] * h^T[k, b]   (W tiles stationary, h^T moving)
The input projection x_t @ W_ih^T is folded into the same PSUM accumulation as a
one-hot matmul against a precomputed table = emb @ W_ih^T (V x 4H): two extra
K-chunks of 128 on top of the 8 H-chunks. Gate biases ride on the ACT engine's
per-partition bias operand. Matmul inputs are bf16; accumulation and cell state
are fp32.
"""

import numpy as np
import ml_dtypes

import concourse.bass as bass
import concourse.tile as tile
from concourse import bacc, mybir
from concourse.bass_utils import run_bass_kernel_spmd

F32 = mybir.dt.float32
BF16 = mybir.dt.bfloat16
AF = mybir.ActivationFunctionType
BFNP = ml_dtypes.bfloat16

L = 64          # time steps
B_FULL = 2048   # full batch
H = 1024        # hidden
E = 50          # char embedding (folded into table)
V = 256         # vocab
NCORES = 8
B = B_FULL // NCORES   # 256 per-core batch
KH = H // 128          # 8 h-feature chunks
KV = V // 128          # 2 one-hot chunks
NJ = KH                # 8 hidden tiles for elementwise
MP = V // 128          # 2 projection out tiles

_BUILD_CACHE: dict = {}


def _build(nsteps: int = L):
    if nsteps in _BUILD_CACHE:
        return _BUILD_CACHE[nsteps]
    nc = bacc.Bacc("TRN2", target_bir_lowering=False, debug=False)

    whhT = nc.dram_tensor("whhT", [H, 4 * H], BF16, kind="ExternalInput").ap()
    tab = nc.dram_tensor("tab", [V, 4 * H], BF16, kind="ExternalInput").ap()
    wpT = nc.dram_tensor("wpT", [H, V], BF16, kind="ExternalInput").ap()
    biasT = nc.dram_tensor("biasT", [128, 4 * H // 128], F32, kind="ExternalInput").ap()
    bprjT = nc.dram_tensor("bprjT", [128, MP], F32, kind="ExternalInput").ap()
    oh = nc.dram_tensor("oh", [L, KV, 128, B], BF16, kind="ExternalInput").ap()
    h0T = nc.dram_tensor("h0T", [H, B], BF16, kind="ExternalInput").ap()
    c0T = nc.dram_tensor("c0T", [H, B], F32, kind="ExternalInput").ap()

    sc = nc.dram_tensor("sc", [L, MP, 128, B], F32, kind="ExternalOutput").ap()
    hT = nc.dram_tensor("hT", [H, B], F32, kind="ExternalOutput").ap()
    cT = nc.dram_tensor("cT", [H, B], F32, kind="ExternalOutput").ap()

    with tile.TileContext(nc) as tc:
        _body(nc, tc, nsteps, whhT, tab, wpT, biasT, bprjT, oh, h0T, c0T, sc, hT, cT)
    nc.compile()
    _BUILD_CACHE[nsteps] = nc
    return nc


def _body(nc, tc, nsteps, whhT, tab, wpT, biasT, bprjT, oh, h0T, c0T, sc, hT, cT):
    from contextlib import ExitStack

    ctx = ExitStack()
    with ctx:
        const = ctx.enter_context(tc.tile_pool(name="const", bufs=1))
        state = ctx.enter_context(tc.tile_pool(name="state", bufs=1))
        ohp = ctx.enter_context(tc.tile_pool(name="ohp", bufs=6))
        gtmp = ctx.enter_context(tc.tile_pool(name="gtmp", bufs=3))
        tcp = ctx.enter_context(tc.tile_pool(name="tcp", bufs=10))
        scp = ctx.enter_context(tc.tile_pool(name="scp", bufs=4))
        pg = ctx.enter_context(tc.tile_pool(name="pg", bufs=6, space="PSUM"))
        pp = ctx.enter_context(tc.tile_pool(name="pp", bufs=2, space="PSUM"))

        # ---- resident weights/bias ----
        whh_sb = []
        for k in range(KH):
            w = const.tile([128, 4 * H], BF16, tag=f"whh{k}", name=f"whh{k}")
            nc.sync.dma_start(w[:], whhT[k * 128:(k + 1) * 128, :])
            whh_sb.append(w)
        tab_sb = []
        for v in range(KV):
            t_ = const.tile([128, 4 * H], BF16, tag=f"tab{v}", name=f"tab{v}")
            nc.sync.dma_start(t_[:], tab[v * 128:(v + 1) * 128, :])
            tab_sb.append(t_)
        wp_sb = []
        for k in range(KH):
            w = const.tile([128, V], BF16, tag=f"wp{k}", name=f"wp{k}")
            nc.sync.dma_start(w[:], wpT[k * 128:(k + 1) * 128, :])
            wp_sb.append(w)
        bias_sb = const.tile([128, 4 * H // 128], F32, tag="bias", name="bias_sb")
        nc.sync.dma_start(bias_sb[:], biasT[:])
        bprj_sb = const.tile([128, MP], F32, tag="bprj", name="bprj_sb")
        nc.sync.dma_start(bprj_sb[:], bprjT[:])

        # ---- recurrent state ----
        c_sb = []
        for j in range(NJ):
            c_ = state.tile([128, B], F32, tag=f"c{j}", name=f"c{j}")
            nc.sync.dma_start(c_[:], c0T[j * 128:(j + 1) * 128, :])
            c_sb.append(c_)
        h_bf = [[], []]
        for p in range(2):
            for j in range(NJ):
                h_ = state.tile([128, B], BF16, tag=f"h{p}_{j}", name=f"h{p}_{j}")
                if p == 0:
                    nc.sync.dma_start(h_[:], h0T[j * 128:(j + 1) * 128, :])
                h_bf[p].append(h_)

        GATE_I, GATE_F, GATE_G, GATE_O = 0, 1, 2, 3

        def gate_psum(m, h_prev, oh_t):
            """Accumulate one 128-row gate tile: [128, B] psum."""
            ps = pg.tile([128, B], F32, tag="ps", name=f"ps_{m}")
            for k in range(KH):
                nc.tensor.matmul(
                    ps[:], whh_sb[k][:, m * 128:(m + 1) * 128], h_prev[k][:],
                    start=(k == 0), stop=False,
                )
            for v in range(KV):
                nc.tensor.matmul(
                    ps[:], tab_sb[v][:, m * 128:(m + 1) * 128], oh_t[v][:],
                    start=False, stop=(v == KV - 1),
                )
            return ps

        for t in range(nsteps):
            h_prev = h_bf[t % 2]
            h_next = h_bf[(t + 1) % 2]

            oh_t = []
            for v in range(KV):
                o_ = ohp.tile([128, B], BF16, tag=f"oh{v}", name=f"oh{v}_{t}")
                nc.sync.dma_start(o_[:], oh[t, v])
                oh_t.append(o_)

            tanh_c = []
            # phase A: i, f, g gates + cell update, per hidden tile j
            for j in range(NJ):
                ps_i = gate_psum(GATE_I * NJ + j, h_prev, oh_t)
                si = gtmp.tile([128, B], F32, tag="si", name=f"si{t}_{j}")
                nc.scalar.activation(
                    si[:], ps_i[:], AF.Sigmoid,
                    bias=bias_sb[:, GATE_I * NJ + j:GATE_I * NJ + j + 1])

                ps_f = gate_psum(GATE_F * NJ + j, h_prev, oh_t)
                sf = gtmp.tile([128, B], F32, tag="sf", name=f"sf{t}_{j}")
                nc.scalar.activation(
                    sf[:], ps_f[:], AF.Sigmoid,
                    bias=bias_sb[:, GATE_F * NJ + j:GATE_F * NJ + j + 1])

                ps_g = gate_psum(GATE_G * NJ + j, h_prev, oh_t)
                tg = gtmp.tile([128, B], F32, tag="tg", name=f"tg{t}_{j}")
                nc.scalar.activation(
                    tg[:], ps_g[:], AF.Tanh,
                    bias=bias_sb[:, GATE_G * NJ + j:GATE_G * NJ + j + 1])

                # c = sf * c + si * tg
                nc.vector.tensor_mul(si[:], si[:], tg[:])
                nc.vector.tensor_mul(c_sb[j][:], sf[:], c_sb[j][:])
                nc.vector.tensor_add(c_sb[j][:], c_sb[j][:], si[:])
                tc_ = tcp.tile([128, B], F32, tag="tc", name=f"tc{t}_{j}")
                nc.scalar.activation(tc_[:], c_sb[j][:], AF.Tanh)
                tanh_c.append(tc_)

            # phase B: o gate + h update
            for j in range(NJ):
                ps_o = gate_psum(GATE_O * NJ + j, h_prev, oh_t)
                so = gtmp.tile([128, B], F32, tag="so", name=f"so{t}_{j}")
                nc.scalar.activation(
                    so[:], ps_o[:], AF.Sigmoid,
                    bias=bias_sb[:, GATE_O * NJ + j:GATE_O * NJ + j + 1])
                nc.vector.tensor_mul(h_next[j][:], so[:], tanh_c[j][:])
                if t == nsteps - 1:
                    hf = gtmp.tile([128, B], F32, tag="hf", name=f"hf{j}")
                    nc.vector.tensor_mul(hf[:], so[:], tanh_c[j][:])
                    nc.sync.dma_start(hT[j * 128:(j + 1) * 128, :], hf[:])
                    nc.sync.dma_start(cT[j * 128:(j + 1) * 128, :], c_sb[j][:])

            # projection: scores_t^T = W_proj h_t (+ b_proj)
            for vt in range(MP):
                psp = pp.tile([128, B], F32, tag="pp", name=f"pp{t}_{vt}")
                for k in range(KH):
                    nc.tensor.matmul(
                        psp[:], wp_sb[k][:, vt * 128:(vt + 1) * 128], h_next[k][:],
                        start=(k == 0), stop=(k == KH - 1),
                    )
                sct = scp.tile([128, B], F32, tag="sct", name=f"sct{t}_{vt}")
                nc.scalar.activation(
                    sct[:], psp[:], AF.Identity, bias=bprj_sb[:, vt:vt + 1])
                nc.sync.dma_start(sc[t, vt], sct[:])


def _prep_inputs(inputs, nsteps: int = L):
    """Host-side shard + repack. Weight-only preprocessing plus one-hot encode."""
    inp = np.asarray(inputs["input"])
    h0 = np.asarray(inputs["h0"], np.float32)
    c0 = np.asarray(inputs["c0"], np.float32)
    emb = np.asarray(inputs["emb"], np.float32)
    W_ih = np.asarray(inputs["W_ih"], np.float32)
    W_hh = np.asarray(inputs["W_hh"], np.float32)
    b = (np.asarray(inputs["b_ih"], np.float32)
         + np.asarray(inputs["b_hh"], np.float32))
    W_proj = np.asarray(inputs["W_proj"], np.float32)
    b_proj = np.asarray(inputs["b_proj"], np.float32)

    whhT = np.ascontiguousarray(W_hh.T).astype(BFNP)             # (H, 4H)
    tabf = emb @ W_ih.T                                          # (V, 4H) f32
    tab = np.ascontiguousarray(tabf).astype(BFNP)
    wpT = np.ascontiguousarray(W_proj.T).astype(BFNP)            # (H, V)
    biasT = np.ascontiguousarray(b.reshape(4 * H // 128, 128).T).astype(np.float32)
    bprjT = np.ascontiguousarray(b_proj.reshape(MP, 128).T).astype(np.float32)

    vvals = np.arange(V, dtype=inp.dtype).reshape(KV, 128)
    in_maps = []
    for c in range(NCORES):
        b0 = c * B
        idx = inp[:nsteps, b0:b0 + B]                            # (L, B)
        ohc = (idx[:, None, None, :] == vvals[None, :, :, None]).astype(BFNP)
        if nsteps < L:
            ohc = np.concatenate(
                [ohc, np.zeros((L - nsteps,) + ohc.shape[1:], BFNP)], axis=0)
        in_maps.append({
            "whhT": whhT, "tab": tab, "wpT": wpT,
            "biasT": biasT, "bprjT": bprjT,
            "oh": np.ascontiguousarray(ohc),
            "h0T": np.ascontiguousarray(h0[0, b0:b0 + B, :].T).astype(BFNP),
            "c0T": np.ascontiguousarray(c0[0, b0:b0 + B, :].T),
        })
    return in_maps


def _assemble(results):
    scores = np.empty((L, B_FULL, V), np.float32)
    hT = np.empty((1, B_FULL, H), np.float32)
    cT = np.empty((1, B_FULL, H), np.float32)
    for c, r in enumerate(results):
        b0 = c * B
        # sc: (L, MP, 128, B) -> (L, B, V)
        scores[:, b0:b0 + B, :] = (
            r["sc"].transpose(0, 3, 1, 2).reshape(L, B, V))
        hT[0, b0:b0 + B, :] = r["hT"].T
        cT[0, b0:b0 + B, :] = r["cT"].T
    return scores, hT, cT


def kernel(**inputs):
    nc = _build(L)
    in_maps = _prep_inputs(inputs, L)
    res = run_bass_kernel_spmd(nc, in_maps, core_ids=list(range(NCORES)))
    return _assemble(res.results)


if __name__ == "__main__":
    data = np.load("/root/problem/data.npz")
    inputs = {k: data[k] for k in
              ["input", "h0", "c0", "emb", "W_ih", "W_hh", "b_ih", "b_hh",
               "W_proj", "b_proj"]}
    out = kernel(**inputs)
    for name, got, exp in zip(
            ["scores", "hT", "cT"], out,
            [data["ref_scores"], data["ref_hT"], data["ref_cT"]]):
        rl2 = np.linalg.norm(got - exp) / np.linalg.norm(exp)
        print(f"{name}: rel_l2={rl2:.3e} absmax={np.abs(got-exp).max():.3e}")


# revision 2
# speedup vs baseline: 1.0141x; 1.0141x over previous
"""CharDecoder LSTM (L=64, B=2048, H=1024, V=256) as a Bass/Tile kernel on 8 trn2 cores.

Strategy: data-parallel over batch (256 per core), weights replicated.
All on-chip state is in [feature, batch] (transposed) layout so the recurrent
matmul needs no transposes:
  gates^T[j, b] = sum_k W_hh^T[k, j] * h^T[k, b]   (W tiles stationary, h^T moving)

The input projection x_t @ W_ih^T + bias is a row lookup into a precomputed
table' = emb @ W_ih^T + (b_ih + b_hh)  (V x 4H). Per step it is fetched with a
single GPSIMD ap_gather (d=32 bf16 words per index) from an SBUF-resident
[128p, 256v, 32jb] arrangement of table', then added to the PSUM gate
pre-activations by DVE. All 4 gates of a hidden chunk share one 2-bank PSUM
tile so the add and the sigmoid/tanh run as wide merged ops. Matmul inputs are
bf16; accumulation and cell state are fp32.
"""

import numpy as np
import ml_dtypes

import concourse.bass as bass
import concourse.tile as tile
from concourse import bacc, mybir
from concourse.bass_utils import run_bass_kernel_spmd

F32 = mybir.dt.float32
BF16 = mybir.dt.bfloat16
I16 = mybir.dt.int16
AF = mybir.ActivationFunctionType
BFNP = ml_dtypes.bfloat16

L = 64          # time steps
B_FULL = 2048   # full batch
H = 1024        # hidden
E = 50          # char embedding (folded into table)
V = 256         # vocab
NCORES = 8
B = B_FULL // NCORES   # 256 per-core batch
KH = H // 128          # 8 h-feature chunks
NJ = KH                # 8 hidden tiles for elementwise
NG = 4 * H // 128      # 32 gate tiles
MP = V // 128          # 2 projection out tiles

_BUILD_CACHE: dict = {}


def _build(nsteps: int = L):
    if nsteps in _BUILD_CACHE:
        return _BUILD_CACHE[nsteps]
    nc = bacc.Bacc("TRN2", target_bir_lowering=False, debug=False)

    whhT = nc.dram_tensor("whhT", [H, 4 * H], BF16, kind="ExternalInput").ap()
    tab3 = nc.dram_tensor("tab3", [128, V * NG], BF16, kind="ExternalInput").ap()
    wpT = nc.dram_tensor("wpT", [H, V], BF16, kind="ExternalInput").ap()
    bprjT = nc.dram_tensor("bprjT", [128, MP], F32, kind="ExternalInput").ap()
    idxs = nc.dram_tensor("idxs", [L, 128, B // 16], I16, kind="ExternalInput").ap()
    h0T = nc.dram_tensor("h0T", [H, B], BF16, kind="ExternalInput").ap()
    c0T = nc.dram_tensor("c0T", [H, B], F32, kind="ExternalInput").ap()

    sc = nc.dram_tensor("sc", [L, MP, 128, B], F32, kind="ExternalOutput").ap()
    hT = nc.dram_tensor("hT", [H, B], F32, kind="ExternalOutput").ap()
    cT = nc.dram_tensor("cT", [H, B], F32, kind="ExternalOutput").ap()

    with tile.TileContext(nc) as tc:
        _body(nc, tc, nsteps, whhT, tab3, wpT, bprjT, idxs, h0T, c0T, sc, hT, cT)
    nc.compile()
    _BUILD_CACHE[nsteps] = nc
    return nc


def _body(nc, tc, nsteps, whhT, tab3, wpT, bprjT, idxs, h0T, c0T, sc, hT, cT):
    from contextlib import ExitStack

    ctx = ExitStack()
    with ctx:
        const = ctx.enter_context(tc.tile_pool(name="const", bufs=1))
        state = ctx.enter_context(tc.tile_pool(name="state", bufs=1))
        idxp = ctx.enter_context(tc.tile_pool(name="idxp", bufs=4))
        gxp = ctx.enter_context(tc.tile_pool(name="gxp", bufs=3))
        tsp = ctx.enter_context(tc.tile_pool(name="tsp", bufs=3))
        tap = ctx.enter_context(tc.tile_pool(name="tap", bufs=3))
        tcp = ctx.enter_context(tc.tile_pool(name="tcp", bufs=10))
        scp = ctx.enter_context(tc.tile_pool(name="scp", bufs=4))
        pg = ctx.enter_context(tc.tile_pool(name="pg", bufs=3, space="PSUM"))
        pp = ctx.enter_context(tc.tile_pool(name="pp", bufs=2, space="PSUM"))

        # ---- resident weights ----
        whh_sb = []
        for k in range(KH):
            w = const.tile([128, 4 * H], BF16, tag=f"whh{k}", name=f"whh{k}")
            nc.sync.dma_start(w[:], whhT[k * 128:(k + 1) * 128, :])
            whh_sb.append(w)
        tab_sb = const.tile([128, V * NG], BF16, tag="tab3", name="tab_sb")
        nc.sync.dma_start(tab_sb[:], tab3[:])
        wp_sb = []
        for k in range(KH):
            w = const.tile([128, V], BF16, tag=f"wp{k}", name=f"wp{k}")
            nc.sync.dma_start(w[:], wpT[k * 128:(k + 1) * 128, :])
            wp_sb.append(w)
        bprj_sb = const.tile([128, MP], F32, tag="bprj", name="bprj_sb")
        nc.sync.dma_start(bprj_sb[:], bprjT[:])

        # ---- recurrent state ----
        c_sb = []
        for j in range(NJ):
            c_ = state.tile([128, B], F32, tag=f"c{j}", name=f"c{j}")
            nc.sync.dma_start(c_[:], c0T[j * 128:(j + 1) * 128, :])
            c_sb.append(c_)
        h_bf = [[], []]
        for p in range(2):
            for j in range(NJ):
                h_ = state.tile([128, B], BF16, tag=f"h{p}_{j}", name=f"h{p}_{j}")
                if p == 0:
                    nc.sync.dma_start(h_[:], h0T[j * 128:(j + 1) * 128, :])
                h_bf[p].append(h_)

        KSEQ = list(range(KH - 1, -1, -1))  # accumulate k descending (h[7] ready first)
        JSEQ = list(range(NJ - 1, -1, -1))  # process hidden chunks descending

        def gather(t):
            """One ap_gather fetching table'[idx_b] for all 32 gate chunks."""
            ix = idxp.tile([128, B // 16], I16, tag="ix", name=f"ix{t}")
            nc.sync.dma_start(ix[:], idxs[t])
            gx = gxp.tile([128, V * NG], BF16, tag="gx", name=f"gx{t}")
            nc.gpsimd.ap_gather(gx[:], tab_sb[:], ix[:], channels=128,
                                num_elems=V, d=NG, num_idxs=B)
            return gx

        gx_q = [gather(0), gather(1)]

        for t in range(nsteps):
            h_prev = h_bf[t % 2]
            h_next = h_bf[(t + 1) % 2]
            gx = gx_q.pop(0)
            # gx view: free index = b*NG + g*NJ + j  ->  [j][g, b] slices
            gxv = gx.rearrange("p (b g j) -> p j g b", b=B, g=4, j=NJ)

            for j in JSEQ:
                ps = pg.tile([128, 4 * B], F32, tag="ps", name=f"ps{t}_{j}")
                for g in range(4):
                    m = g * NJ + j
                    for ki, k in enumerate(KSEQ):
                        nc.tensor.matmul(
                            ps[:, g * B:(g + 1) * B],
                            whh_sb[k][:, m * 128:(m + 1) * 128], h_prev[k][:],
                            start=(ki == 0), stop=(ki == KH - 1),
                        )
                # pre-activations: ts = psum + table-row (bias already folded in)
                ts = tsp.tile([128, 4 * B], F32, tag="ts", name=f"ts{t}_{j}")
                nc.vector.tensor_add(ts[:], ps[:], gxv[:, j])
                ta = tap.tile([128, 4 * B], F32, tag="ta", name=f"ta{t}_{j}")
                nc.scalar.activation(ta[:, 0:2 * B], ts[:, 0:2 * B], AF.Sigmoid)
                nc.scalar.activation(ta[:, 2 * B:3 * B], ts[:, 2 * B:3 * B], AF.Tanh)
                nc.scalar.activation(ta[:, 3 * B:4 * B], ts[:, 3 * B:4 * B], AF.Sigmoid)
                si, sf = ta[:, 0:B], ta[:, B:2 * B]
                tg, so = ta[:, 2 * B:3 * B], ta[:, 3 * B:4 * B]
                # c = sf * c + si * tg ; h = so * tanh(c)
                nc.vector.tensor_mul(si, si, tg)
                nc.vector.tensor_mul(c_sb[j][:], sf, c_sb[j][:])
                nc.vector.tensor_add(c_sb[j][:], c_sb[j][:], si)
                tc_ = tcp.tile([128, B], F32, tag="tc", name=f"tc{t}_{j}")
                nc.scalar.activation(tc_[:], c_sb[j][:], AF.Tanh)
                nc.vector.tensor_mul(h_next[j][:], so, tc_[:])
                if t == nsteps - 1:
                    hf = tcp.tile([128, B], F32, tag="hf", name=f"hf{j}")
                    nc.vector.tensor_mul(hf[:], so, tc_[:])
                    nc.sync.dma_start(hT[j * 128:(j + 1) * 128, :], hf[:])
                    nc.sync.dma_start(cT[j * 128:(j + 1) * 128, :], c_sb[j][:])

            # prefetch gather two steps ahead (gpsimd runs it while PE works)
            if t + 2 < nsteps:
                gx_q.append(gather(t + 2))

            # projection: scores_t^T = W_proj h_t + b_proj
            for vt in range(MP):
                psp = pp.tile([128, B], F32, tag="pp", name=f"pp{t}_{vt}")
                for ki, k in enumerate(KSEQ):
                    nc.tensor.matmul(
                        psp[:], wp_sb[k][:, vt * 128:(vt + 1) * 128], h_next[k][:],
                        start=(ki == 0), stop=(ki == KH - 1),
                    )
                sct = scp.tile([128, B], F32, tag="sct", name=f"sct{t}_{vt}")
                nc.scalar.activation(
                    sct[:], psp[:], AF.Identity, bias=bprj_sb[:, vt:vt + 1])
                nc.sync.dma_start(sc[t, vt], sct[:])


def _prep_inputs(inputs, nsteps: int = L):
    """Host-side shard + repack: weight transposes, bias-folded gather table,
    per-step wrapped index tiles."""
    inp = np.asarray(inputs["input"])
    h0 = np.asarray(inputs["h0"], np.float32)
    c0 = np.asarray(inputs["c0"], np.float32)
    emb = np.asarray(inputs["emb"], np.float32)
    W_ih = np.asarray(inputs["W_ih"], np.float32)
    W_hh = np.asarray(inputs["W_hh"], np.float32)
    b = (np.asarray(inputs["b_ih"], np.float32)
         + np.asarray(inputs["b_hh"], np.float32))
    W_proj = np.asarray(inputs["W_proj"], np.float32)
    b_proj = np.asarray(inputs["b_proj"], np.float32)

    whhT = np.ascontiguousarray(W_hh.T).astype(BFNP)             # (H, 4H)
    table = emb @ W_ih.T + b                                     # (V, 4H) f32
    # tab3[p, v, jb] = table[v, jb*128 + p]
    tab3 = np.ascontiguousarray(
        table.reshape(V, NG, 128).transpose(2, 0, 1).reshape(128, V * NG)
    ).astype(BFNP)
    wpT = np.ascontiguousarray(W_proj.T).astype(BFNP)            # (H, V)
    bprjT = np.ascontiguousarray(b_proj.reshape(MP, 128).T).astype(np.float32)

    in_maps = []
    for c in range(NCORES):
        b0 = c * B
        idx = inp[:nsteps, b0:b0 + B].astype(np.int16)           # (L, B)
        # wrapped layout: ix[t, p, f] = idx[t, f*16 + p%16], replicated per 16p
        wrapped = idx.reshape(nsteps, B // 16, 16).transpose(0, 2, 1)  # (L,16,B/16)
        ixt = np.tile(wrapped, (1, 8, 1))                        # (L, 128, B/16)
        if nsteps < L:
            ixt = np.concatenate(
                [ixt, np.zeros((L - nsteps,) + ixt.shape[1:], np.int16)], axis=0)
        in_maps.append({
            "whhT": whhT, "tab3": tab3, "wpT": wpT, "bprjT": bprjT,
            "idxs": np.ascontiguousarray(ixt),
            "h0T": np.ascontiguousarray(h0[0, b0:b0 + B, :].T).astype(BFNP),
            "c0T": np.ascontiguousarray(c0[0, b0:b0 + B, :].T),
        })
    return in_maps


def _assemble(results):
    scores = np.empty((L, B_FULL, V), np.float32)
    hT = np.empty((1, B_FULL, H), np.float32)
    cT = np.empty((1, B_FULL, H), np.float32)
    for c, r in enumerate(results):
        b0 = c * B
        scores[:, b0:b0 + B, :] = r["sc"].transpose(0, 3, 1, 2).reshape(L, B, V)
        hT[0, b0:b0 + B, :] = r["hT"].T
        cT[0, b0:b0 + B, :] = r["cT"].T
    return scores, hT, cT


def kernel(**inputs):
    nc = _build(L)
    in_maps = _prep_inputs(inputs, L)
    res = run_bass_kernel_spmd(nc, in_maps, core_ids=list(range(NCORES)))
    return _assemble(res.results)


if __name__ == "__main__":
    data = np.load("/root/problem/data.npz")
    inputs = {k: data[k] for k in
              ["input", "h0", "c0", "emb", "W_ih", "W_hh", "b_ih", "b_hh",
               "W_proj", "b_proj"]}
    out = kernel(**inputs)
    for name, got, exp in zip(
            ["scores", "hT", "cT"], out,
            [data["ref_scores"], data["ref_hT"], data["ref_cT"]]):
        rl2 = np.linalg.norm(got - exp) / np.linalg.norm(exp)
        print(f"{name}: rel_l2={rl2:.3e} absmax={np.abs(got-exp).max():.3e}")


# revision 3
# speedup vs baseline: 1.1973x; 1.1807x over previous
"""CharDecoder LSTM (L=64, B=2048, H=1024, V=256) as a Bass/Tile kernel on 8 trn2 cores.

Strategy: data-parallel over batch (256 per core), weights replicated.
All on-chip state is in [feature, batch] (transposed) layout so the recurrent
matmul needs no transposes:
  gates^T[j, b] = sum_k W_hh^T[k, j] * h^T[k, b]   (W tiles stationary, h^T moving)

The input projection x_t @ W_ih^T + bias is a row lookup into a precomputed
table' = emb @ W_ih^T + (b_ih + b_hh)  (V x 4H), fetched per step straight from
HBM with one dma_gather(transpose=True): output lands as [128p, 32chunk, 256b]
with batch contiguous, so a single wide DVE add merges it into each hidden
chunk's 4-gate PSUM tile. All 4 gates of a hidden chunk share one 2-bank PSUM
tile so the add and the sigmoid/tanh run as wide merged ops. Matmul inputs are
bf16; accumulation and cell state are fp32.
"""

import numpy as np
import ml_dtypes

import concourse.bass as bass
import concourse.tile as tile
from concourse import bacc, mybir
from concourse.bass_utils import run_bass_kernel_spmd

F32 = mybir.dt.float32
BF16 = mybir.dt.bfloat16
I16 = mybir.dt.int16
AF = mybir.ActivationFunctionType
BFNP = ml_dtypes.bfloat16

L = 64          # time steps
B_FULL = 2048   # full batch
H = 1024        # hidden
E = 50          # char embedding (folded into table)
V = 256         # vocab
NCORES = 8
B = B_FULL // NCORES   # 256 per-core batch
KH = H // 128          # 8 h-feature chunks
NJ = KH                # 8 hidden tiles for elementwise
NG = 4 * H // 128      # 32 gate tiles
MP = V // 128          # 2 projection out tiles

_BUILD_CACHE: dict = {}


def _build(nsteps: int = L):
    if nsteps in _BUILD_CACHE:
        return _BUILD_CACHE[nsteps]
    nc = bacc.Bacc("TRN2", target_bir_lowering=False, debug=False)

    whhT = nc.dram_tensor("whhT", [H, 4 * H], BF16, kind="ExternalInput").ap()
    tabg = nc.dram_tensor("tabg", [V, 4 * H], BF16, kind="ExternalInput").ap()
    wpT = nc.dram_tensor("wpT", [H, V], BF16, kind="ExternalInput").ap()
    bprjT = nc.dram_tensor("bprjT", [128, MP], F32, kind="ExternalInput").ap()
    idxs = nc.dram_tensor("idxs", [L, 128, B // 16], I16, kind="ExternalInput").ap()
    h0T = nc.dram_tensor("h0T", [H, B], BF16, kind="ExternalInput").ap()
    c0T = nc.dram_tensor("c0T", [H, B], F32, kind="ExternalInput").ap()

    sc = nc.dram_tensor("sc", [L, MP, 128, B], F32, kind="ExternalOutput").ap()
    hT = nc.dram_tensor("hT", [H, B], F32, kind="ExternalOutput").ap()
    cT = nc.dram_tensor("cT", [H, B], F32, kind="ExternalOutput").ap()

    with tile.TileContext(nc) as tc:
        _body(nc, tc, nsteps, whhT, tabg, wpT, bprjT, idxs, h0T, c0T, sc, hT, cT)
    nc.compile()
    _BUILD_CACHE[nsteps] = nc
    return nc


def _body(nc, tc, nsteps, whhT, tabg, wpT, bprjT, idxs, h0T, c0T, sc, hT, cT):
    from contextlib import ExitStack

    ctx = ExitStack()
    with ctx:
        const = ctx.enter_context(tc.tile_pool(name="const", bufs=1))
        state = ctx.enter_context(tc.tile_pool(name="state", bufs=1))
        idxp = ctx.enter_context(tc.tile_pool(name="idxp", bufs=4))
        gxp = ctx.enter_context(tc.tile_pool(name="gxp", bufs=3))
        tsp = ctx.enter_context(tc.tile_pool(name="tsp", bufs=3))
        tap = ctx.enter_context(tc.tile_pool(name="tap", bufs=3))
        tcp = ctx.enter_context(tc.tile_pool(name="tcp", bufs=10))
        scp = ctx.enter_context(tc.tile_pool(name="scp", bufs=4))
        pg = ctx.enter_context(tc.tile_pool(name="pg", bufs=3, space="PSUM"))
        pp = ctx.enter_context(tc.tile_pool(name="pp", bufs=2, space="PSUM"))

        # ---- resident weights ----
        whh_sb = []
        for k in range(KH):
            w = const.tile([128, 4 * H], BF16, tag=f"whh{k}", name=f"whh{k}")
            nc.sync.dma_start(w[:], whhT[k * 128:(k + 1) * 128, :])
            whh_sb.append(w)
        wp_sb = []
        for k in range(KH):
            w = const.tile([128, V], BF16, tag=f"wp{k}", name=f"wp{k}")
            nc.sync.dma_start(w[:], wpT[k * 128:(k + 1) * 128, :])
            wp_sb.append(w)
        bprj_sb = const.tile([128, MP], F32, tag="bprj", name="bprj_sb")
        nc.sync.dma_start(bprj_sb[:], bprjT[:])

        # ---- recurrent state ----
        c_sb = []
        for j in range(NJ):
            c_ = state.tile([128, B], F32, tag=f"c{j}", name=f"c{j}")
            nc.sync.dma_start(c_[:], c0T[j * 128:(j + 1) * 128, :])
            c_sb.append(c_)
        h_bf = [[], []]
        for p in range(2):
            for j in range(NJ):
                h_ = state.tile([128, B], BF16, tag=f"h{p}_{j}", name=f"h{p}_{j}")
                if p == 0:
                    nc.sync.dma_start(h_[:], h0T[j * 128:(j + 1) * 128, :])
                h_bf[p].append(h_)

        KSEQ = list(range(KH - 1, -1, -1))  # accumulate k descending (h[7] ready first)
        JSEQ = list(range(NJ - 1, -1, -1))  # process hidden chunks descending

        def gather(t):
            """table'[idx_b] for all 32 gate chunks -> [128, 32, 256], b contiguous."""
            ix = idxp.tile([128, B // 16], I16, tag="ix", name=f"ix{t}")
            nc.sync.dma_start(ix[:], idxs[t])
            gx = gxp.tile([128, NG, B], BF16, tag="gx", name=f"gx{t}")
            nc.gpsimd.dma_gather(gx[:], tabg[:], ix[:], num_idxs=B,
                                 num_idxs_reg=B, elem_size=4 * H, transpose=True)
            return gx

        gx_q = [gather(0), gather(1)]

        for t in range(nsteps):
            h_prev = h_bf[t % 2]
            h_next = h_bf[(t + 1) % 2]
            gx = gx_q.pop(0)

            for j in JSEQ:
                ps = pg.tile([128, 4 * B], F32, tag="ps", name=f"ps{t}_{j}")
                for g in range(4):
                    m = g * NJ + j
                    for ki, k in enumerate(KSEQ):
                        nc.tensor.matmul(
                            ps[:, g * B:(g + 1) * B],
                            whh_sb[k][:, m * 128:(m + 1) * 128], h_prev[k][:],
                            start=(ki == 0), stop=(ki == KH - 1),
                        )
                # pre-activations: ts = psum + table-row (bias already folded in);
                # gx[:, j::NJ, :] selects gate chunks (i,f,g,o) of hidden chunk j
                ts = tsp.tile([128, 4 * B], F32, tag="ts", name=f"ts{t}_{j}")
                nc.vector.tensor_add(
                    ts.rearrange("p (g b) -> p g b", g=4), ps.rearrange("p (g b) -> p g b", g=4),
                    gx[:, j::NJ, :])
                ta = tap.tile([128, 4 * B], F32, tag="ta", name=f"ta{t}_{j}")
                nc.scalar.activation(ta[:, 0:2 * B], ts[:, 0:2 * B], AF.Sigmoid)
                nc.scalar.activation(ta[:, 2 * B:3 * B], ts[:, 2 * B:3 * B], AF.Tanh)
                nc.scalar.activation(ta[:, 3 * B:4 * B], ts[:, 3 * B:4 * B], AF.Sigmoid)
                si, sf = ta[:, 0:B], ta[:, B:2 * B]
                tg, so = ta[:, 2 * B:3 * B], ta[:, 3 * B:4 * B]
                # c = sf * c + si * tg ; h = so * tanh(c)
                nc.vector.tensor_mul(si, si, tg)
                nc.vector.tensor_mul(c_sb[j][:], sf, c_sb[j][:])
                nc.vector.tensor_add(c_sb[j][:], c_sb[j][:], si)
                tc_ = tcp.tile([128, B], F32, tag="tc", name=f"tc{t}_{j}")
                nc.scalar.activation(tc_[:], c_sb[j][:], AF.Tanh)
                nc.vector.tensor_mul(h_next[j][:], so, tc_[:])
                if t == nsteps - 1:
                    hf = tcp.tile([128, B], F32, tag="hf", name=f"hf{j}")
                    nc.vector.tensor_mul(hf[:], so, tc_[:])
                    nc.sync.dma_start(hT[j * 128:(j + 1) * 128, :], hf[:])
                    nc.sync.dma_start(cT[j * 128:(j + 1) * 128, :], c_sb[j][:])

            # prefetch gather two steps ahead (runs on DMA queues while PE works)
            if t + 2 < nsteps:
                gx_q.append(gather(t + 2))

            # projection: scores_t^T = W_proj h_t + b_proj
            for vt in range(MP):
                psp = pp.tile([128, B], F32, tag="pp", name=f"pp{t}_{vt}")
                for ki, k in enumerate(KSEQ):
                    nc.tensor.matmul(
                        psp[:], wp_sb[k][:, vt * 128:(vt + 1) * 128], h_next[k][:],
                        start=(ki == 0), stop=(ki == KH - 1),
                    )
                sct = scp.tile([128, B], F32, tag="sct", name=f"sct{t}_{vt}")
                nc.scalar.activation(
                    sct[:], psp[:], AF.Identity, bias=bprj_sb[:, vt:vt + 1])
                nc.sync.dma_start(sc[t, vt], sct[:])


def _prep_inputs(inputs, nsteps: int = L):
    """Host-side shard + repack: weight transposes, bias-folded gather table,
    per-step wrapped index tiles."""
    inp = np.asarray(inputs["input"])
    h0 = np.asarray(inputs["h0"], np.float32)
    c0 = np.asarray(inputs["c0"], np.float32)
    emb = np.asarray(inputs["emb"], np.float32)
    W_ih = np.asarray(inputs["W_ih"], np.float32)
    W_hh = np.asarray(inputs["W_hh"], np.float32)
    b = (np.asarray(inputs["b_ih"], np.float32)
         + np.asarray(inputs["b_hh"], np.float32))
    W_proj = np.asarray(inputs["W_proj"], np.float32)
    b_proj = np.asarray(inputs["b_proj"], np.float32)

    whhT = np.ascontiguousarray(W_hh.T).astype(BFNP)             # (H, 4H)
    tabg = np.ascontiguousarray(emb @ W_ih.T + b).astype(BFNP)   # (V, 4H)
    wpT = np.ascontiguousarray(W_proj.T).astype(BFNP)            # (H, V)
    bprjT = np.ascontiguousarray(b_proj.reshape(MP, 128).T).astype(np.float32)

    in_maps = []
    for c in range(NCORES):
        b0 = c * B
        idx = inp[:nsteps, b0:b0 + B].astype(np.int16)           # (L, B)
        # wrapped layout: ix[t, p, f] = idx[t, f*16 + p%16], replicated per 16p
        wrapped = idx.reshape(nsteps, B // 16, 16).transpose(0, 2, 1)  # (L,16,B/16)
        ixt = np.tile(wrapped, (1, 8, 1))                        # (L, 128, B/16)
        if nsteps < L:
            ixt = np.concatenate(
                [ixt, np.zeros((L - nsteps,) + ixt.shape[1:], np.int16)], axis=0)
        in_maps.append({
            "whhT": whhT, "tabg": tabg, "wpT": wpT, "bprjT": bprjT,
            "idxs": np.ascontiguousarray(ixt),
            "h0T": np.ascontiguousarray(h0[0, b0:b0 + B, :].T).astype(BFNP),
            "c0T": np.ascontiguousarray(c0[0, b0:b0 + B, :].T),
        })
    return in_maps


def _assemble(results):
    scores = np.empty((L, B_FULL, V), np.float32)
    hT = np.empty((1, B_FULL, H), np.float32)
    cT = np.empty((1, B_FULL, H), np.float32)
    for c, r in enumerate(results):
        b0 = c * B
        scores[:, b0:b0 + B, :] = r["sc"].transpose(0, 3, 1, 2).reshape(L, B, V)
        hT[0, b0:b0 + B, :] = r["hT"].T
        cT[0, b0:b0 + B, :] = r["cT"].T
    return scores, hT, cT


def kernel(**inputs):
    nc = _build(L)
    in_maps = _prep_inputs(inputs, L)
    res = run_bass_kernel_spmd(nc, in_maps, core_ids=list(range(NCORES)))
    return _assemble(res.results)


if __name__ == "__main__":
    data = np.load("/root/problem/data.npz")
    inputs = {k: data[k] for k in
              ["input", "h0", "c0", "emb", "W_ih", "W_hh", "b_ih", "b_hh",
               "W_proj", "b_proj"]}
    out = kernel(**inputs)
    for name, got, exp in zip(
            ["scores", "hT", "cT"], out,
            [data["ref_scores"], data["ref_hT"], data["ref_cT"]]):
        rl2 = np.linalg.norm(got - exp) / np.linalg.norm(exp)
        print(f"{name}: rel_l2={rl2:.3e} absmax={np.abs(got-exp).max():.3e}")
